# revision 7
# baseline (speedup 1.0000x reference)
"""Trainium2 Bass kernel for nn_DiscountedTypeLoss.

Math: the reference computes f = features @ W.T + b per token, then per-class
(masked by labels) sums of f, then a tiny 16x16 cosine/rank-discount softmax
loss. Since f is linear in features, the per-class sums of f equal
(per-class sums of features) @ W.T + counts * b. So the device kernel only
needs the per-class feature sums [16, 1024] + counts — a one-hot weighted
reduction over 131072 tokens, which is purely memory-bound (512 MB of
features traffic).

Sharding: data-parallel over tokens — each of the 8 cores reduces 4 of the
32 batches (16384 tokens, 64 MB). Per core the kernel streams [128, 1024]
token tiles and accumulates onehot(labels)^T @ features into PSUM via the
tensor engine. The host sums the 8 partial [16, 1024] results, computes
counts with bincount, and finishes the tiny 16x16 math in float64.
"""

import numpy as np

import concourse.bass as bass
import concourse.tile as tile
from concourse import bacc, mybir
from concourse.bass_utils import run_bass_kernel_spmd

N_CORES = 8
B, S, H = 32, 4096, 1024
C = 16               # NUM_TAGS
TOK = (B // N_CORES) * S   # tokens per core = 16384
P = 128
NTILES = TOK // P    # 128
TEMPERATURE = 0.3
EPS = 1e-8

_nc_cache = {}


def build_nc(mm_dtype=mybir.dt.float32r, tiles_per_dma=2, bufs=8):
    nc = bacc.Bacc("TRN2", target_bir_lowering=False, debug=False)
    feats = nc.dram_tensor("feats", [TOK, H], mm_dtype,
                           kind="ExternalInput").ap()
    labs = nc.dram_tensor("labs", [P, NTILES], mybir.dt.float32,
                          kind="ExternalInput").ap()
    sums_out = nc.dram_tensor("sums", [C, H], mybir.dt.float32,
                              kind="ExternalOutput").ap()

    J = tiles_per_dma
    with tile.TileContext(nc) as tc:
        with tc.tile_pool(name="fpool", bufs=bufs) as fpool, \
             tc.tile_pool(name="small", bufs=1) as spool, \
             tc.tile_pool(name="oh", bufs=4) as ohpool, \
             tc.tile_pool(name="acc", bufs=1, space="PSUM") as ppool, \
             tc.tile_pool(name="outp", bufs=1) as outpool:
            lab_sb = spool.tile([P, NTILES], mybir.dt.float32)
            nc.sync.dma_start(out=lab_sb, in_=labs)
            iota = spool.tile([P, C], mybir.dt.float32)
            nc.gpsimd.iota(iota, pattern=[[1, C]], base=0, channel_multiplier=0,
                           allow_small_or_imprecise_dtypes=True)

            psums = [ppool.tile([C, 512], mybir.dt.float32, name=f"psum{h}",
                                tag=f"psum{h}")
                     for h in range(2)]
            for ic in range(NTILES // J):
                ft = fpool.tile([P, J, H], mm_dtype)
                src = feats[ic * J * P:(ic + 1) * J * P, :].rearrange(
                    "(j p) h -> p j h", p=P)
                nc.sync.dma_start(out=ft, in_=src)
                for j in range(J):
                    i = ic * J + j
                    oh = ohpool.tile([P, C], mm_dtype)
                    nc.vector.tensor_scalar(
                        out=oh, in0=iota, scalar1=lab_sb[:, i:i + 1],
                        scalar2=None, op0=mybir.AluOpType.is_equal)
                    for half in range(2):
                        nc.tensor.matmul(
                            psums[half],
                            lhsT=oh,
                            rhs=ft[:, j, half * 512:(half + 1) * 512],
                            start=(i == 0), stop=(i == NTILES - 1))

            out_sb = outpool.tile([C, H], mybir.dt.float32)
            for half in range(2):
                nc.vector.tensor_copy(
                    out=out_sb[:, half * 512:(half + 1) * 512], in_=psums[half])
            nc.sync.dma_start(out=sums_out, in_=out_sb)

    nc.compile()
    return nc


def get_nc():
    if "nc" not in _nc_cache:
        _nc_cache["nc"] = build_nc()
    return _nc_cache["nc"]


def _final_loss(S_feat, counts, W, b, proto):
    """Tiny 16x16 tail of the loss, in float64 (matches fp32 reference to ~1e-8)."""
    dt = np.float64
    W = W.astype(dt)
    b = b.astype(dt)
    proto = proto.astype(dt)
    sums = S_feat @ W.T + counts[:, None] * b[None, :]
    means = sums / np.maximum(counts, 1.0)[:, None]
    mn = np.maximum(np.linalg.norm(means, axis=1), EPS)
    pn = np.maximum(np.linalg.norm(proto, axis=1), EPS)
    cos_mp = (means @ proto.T) / (mn[:, None] * pn[None, :])
    all_pair = -(1.0 - cos_mp) / TEMPERATURE
    sim = (proto @ proto.T) / (pn[:, None] * pn[None, :])
    order = np.argsort(-sim, axis=1, kind="stable")
    rank = np.argsort(order, axis=1, kind="stable")
    discount = np.log2(rank.astype(dt) + 2.0)
    logits = all_pair / discount
    mx = logits.max(axis=1, keepdims=True)
    lse = np.log(np.exp(logits - mx).sum(axis=1)) + mx[:, 0]
    losses = -(np.diag(logits) - lse)
    valid = counts > 0
    return np.sum(np.where(valid, losses, 0.0)) / C


def run_device(features, labels, trace=False):
    feats = np.ascontiguousarray(np.asarray(features, dtype=np.float32)).reshape(
        N_CORES, TOK, H)
    labs = np.asarray(labels, dtype=np.int32).reshape(N_CORES, TOK)
    in_maps = []
    for c in range(N_CORES):
        lab2d = np.ascontiguousarray(labs[c].reshape(NTILES, P).T.astype(np.float32))
        in_maps.append({"feats": feats[c], "labs": lab2d})
    nc = get_nc()
    res = run_bass_kernel_spmd(nc, in_maps, core_ids=list(range(N_CORES)),
                               trace=trace)
    S_feat = np.zeros((C, H), np.float64)
    for m in res.results:
        S_feat += m["sums"].astype(np.float64)
    return S_feat, res


def kernel(features, labels, W, b, proto):
    labels = np.asarray(labels, dtype=np.int32)
    S_feat, _ = run_device(features, labels)
    counts = np.bincount(labels.ravel(), minlength=C).astype(np.float64)
    loss = _final_loss(S_feat, counts,
                       np.asarray(W, np.float32), np.asarray(b, np.float32),
                       np.asarray(proto, np.float32))
    return np.array([loss], dtype=np.float32)


# revision 8
# speedup vs baseline: 1.1550x; 1.1550x over previous
"""Trainium2 Bass kernel for nn_DiscountedTypeLoss.

Math: the reference computes f = features @ W.T + b per token, then per-class
(masked by labels) sums of f, then a tiny 16x16 cosine/rank-discount softmax
loss. Since f is linear in features, the per-class sums of f equal
(per-class sums of features) @ W.T + counts * b. So the device kernel only
needs the per-class feature sums [16, 1024] + counts — a one-hot weighted
reduction over 131072 tokens, which is purely memory-bound (512 MB of
features traffic).

Sharding: data-parallel over tokens — each of the 8 cores reduces 4 of the
32 batches (16384 tokens, 64 MB). Per core the kernel streams [128, 1024]
token tiles and accumulates onehot(labels)^T @ features into PSUM via the
tensor engine. The host sums the 8 partial [16, 1024] results, computes
counts with bincount, and finishes the tiny 16x16 math in float64.
"""

import numpy as np

import concourse.bass as bass
import concourse.tile as tile
from concourse import bacc, mybir
from concourse.bass_utils import run_bass_kernel_spmd

N_CORES = 8
B, S, H = 32, 4096, 1024
C = 16               # NUM_TAGS
TOK = (B // N_CORES) * S   # tokens per core = 16384
P = 128
NTILES = TOK // P    # 128
TEMPERATURE = 0.3
EPS = 1e-8

_nc_cache = {}
IOTA_NP = np.ascontiguousarray(
    np.broadcast_to(np.arange(16, dtype=np.float32)[None, :], (128, 16)))


def build_nc(mm_dtype=mybir.dt.float32r, tiles_per_dma=2, bufs=8):
    nc = bacc.Bacc("TRN2", target_bir_lowering=False, debug=False)
    feats = nc.dram_tensor("feats", [TOK, H], mm_dtype,
                           kind="ExternalInput").ap()
    labs = nc.dram_tensor("labs", [P, NTILES], mybir.dt.float32,
                          kind="ExternalInput").ap()
    iota_in = nc.dram_tensor("iota", [P, C], mybir.dt.float32,
                             kind="ExternalInput").ap()
    sums_out = nc.dram_tensor("sums", [C, H], mybir.dt.float32,
                              kind="ExternalOutput").ap()

    J = tiles_per_dma
    with tile.TileContext(nc) as tc:
        with tc.tile_pool(name="fpool", bufs=bufs) as fpool, \
             tc.tile_pool(name="small", bufs=1) as spool, \
             tc.tile_pool(name="oh", bufs=4) as ohpool, \
             tc.tile_pool(name="acc", bufs=1, space="PSUM") as ppool, \
             tc.tile_pool(name="outp", bufs=1) as outpool:
            lab_sb = spool.tile([P, NTILES], mybir.dt.float32)
            nc.sync.dma_start(out=lab_sb, in_=labs)
            iota = spool.tile([P, C], mybir.dt.float32)
            nc.scalar.dma_start(out=iota, in_=iota_in)

            psums = [ppool.tile([C, 512], mybir.dt.float32, name=f"psum{h}",
                                tag=f"psum{h}")
                     for h in range(2)]
            for ic in range(NTILES // J):
                ft = fpool.tile([P, J, H], mm_dtype)
                src = feats[ic * J * P:(ic + 1) * J * P, :].rearrange(
                    "(j p) h -> p j h", p=P)
                eng = nc.sync if ic % 2 == 0 else nc.scalar
                eng.dma_start(out=ft, in_=src)
                for j in range(J):
                    i = ic * J + j
                    oh = ohpool.tile([P, C], mm_dtype)
                    nc.vector.tensor_scalar(
                        out=oh, in0=iota, scalar1=lab_sb[:, i:i + 1],
                        scalar2=None, op0=mybir.AluOpType.is_equal)
                    for half in range(2):
                        nc.tensor.matmul(
                            psums[half],
                            lhsT=oh,
                            rhs=ft[:, j, half * 512:(half + 1) * 512],
                            start=(i == 0), stop=(i == NTILES - 1))

            out_sb = outpool.tile([C, H], mybir.dt.float32)
            for half in range(2):
                nc.vector.tensor_copy(
                    out=out_sb[:, half * 512:(half + 1) * 512], in_=psums[half])
            nc.sync.dma_start(out=sums_out, in_=out_sb)

    nc.compile()
    return nc


def get_nc():
    if "nc" not in _nc_cache:
        _nc_cache["nc"] = build_nc()
    return _nc_cache["nc"]


def _final_loss(S_feat, counts, W, b, proto):
    """Tiny 16x16 tail of the loss, in float64 (matches fp32 reference to ~1e-8)."""
    dt = np.float64
    W = W.astype(dt)
    b = b.astype(dt)
    proto = proto.astype(dt)
    sums = S_feat @ W.T + counts[:, None] * b[None, :]
    means = sums / np.maximum(counts, 1.0)[:, None]
    mn = np.maximum(np.linalg.norm(means, axis=1), EPS)
    pn = np.maximum(np.linalg.norm(proto, axis=1), EPS)
    cos_mp = (means @ proto.T) / (mn[:, None] * pn[None, :])
    all_pair = -(1.0 - cos_mp) / TEMPERATURE
    sim = (proto @ proto.T) / (pn[:, None] * pn[None, :])
    order = np.argsort(-sim, axis=1, kind="stable")
    rank = np.argsort(order, axis=1, kind="stable")
    discount = np.log2(rank.astype(dt) + 2.0)
    logits = all_pair / discount
    mx = logits.max(axis=1, keepdims=True)
    lse = np.log(np.exp(logits - mx).sum(axis=1)) + mx[:, 0]
    losses = -(np.diag(logits) - lse)
    valid = counts > 0
    return np.sum(np.where(valid, losses, 0.0)) / C


def run_device(features, labels, trace=False):
    feats = np.ascontiguousarray(np.asarray(features, dtype=np.float32)).reshape(
        N_CORES, TOK, H)
    labs = np.asarray(labels, dtype=np.int32).reshape(N_CORES, TOK)
    in_maps = []
    for c in range(N_CORES):
        lab2d = np.ascontiguousarray(labs[c].reshape(NTILES, P).T.astype(np.float32))
        in_maps.append({"feats": feats[c], "labs": lab2d, "iota": IOTA_NP})
    nc = get_nc()
    res = run_bass_kernel_spmd(nc, in_maps, core_ids=list(range(N_CORES)),
                               trace=trace)
    S_feat = np.zeros((C, H), np.float64)
    for m in res.results:
        S_feat += m["sums"].astype(np.float64)
    return S_feat, res


def kernel(features, labels, W, b, proto):
    labels = np.asarray(labels, dtype=np.int32)
    S_feat, _ = run_device(features, labels)
    counts = np.bincount(labels.ravel(), minlength=C).astype(np.float64)
    loss = _final_loss(S_feat, counts,
                       np.asarray(W, np.float32), np.asarray(b, np.float32),
                       np.asarray(proto, np.float32))
    return np.array([loss], dtype=np.float32)
